# revision 2
# baseline (speedup 1.0000x reference)
"""Cross-attention block (q from z_hsi, k/v from z_msi, softmax over 6400
pixels, residual + gamma) on 8 Trainium2 NeuronCores.

Sharding: the (batch=2, N=6400) query-pixel space is split into 8 shards of
1600 pixels (4 shards per batch element). Each core computes its shard's
attention output against the full key/value set of its batch element; the
host slices inputs and concatenates outputs (no device collectives).

Math restructuring vs the naive form:
  * softmax over j is invariant to adding any per-i constant, so the K bias
    (bk) is dropped entirely, and
      E[j,i] = K[:,j]^T Q[:,i]  ==  zm[:,j]^T (Wk^T Wq zq + Wk^T bq)[:,i]
    so K and Q are never materialized: one [64 x 1600] "QK" projection
    (host precomputes Wq^T Wk and Wk^T bq) replaces both.
  * The V bias folds out of the attention matmul:  (V+bv) P = V P + bv * d,
    so after normalizing by d it becomes "+ gamma*bv" on the residual.
  * E matmuls run in float32r (TF32-like, full PE rate at >=256-wide
    outputs).  Everything downstream of exp runs in bfloat16: P tiles, the
    d reduction tree, V^T and the PV matmuls.  bf16 matmuls are 1 cyc/col
    at ANY width (so the 128-wide V^T projections dodge f32r's 4x narrow-
    output penalty), and bf16 DVE ops are eligible for the 2x/4x modes.
  * d[i] = sum_j exp: a 3-level bf16 DVE tree reduces 8 exp tiles to one
    [128 x 400] partial, then a single accumulating ones-matmul per 8 tiles
    does the partition-axis sum (PE denominator cost: 40k -> 11k cycles).
  * exp runs on ACT straight out of PSUM in [128, 2x400] strided batches;
    PV matmuls trail three exp-groups behind (software pipeline).  The
    gamma/d broadcast runs on the otherwise-idle GPSIMD.  ACT is the
    critical engine (~10.2M exps/core at 1 elem/lane/cycle); PE work is
    sized to just undercut it so the two stay overlapped.
"""
import sys

sys.path.insert(0, "/opt/trn_rl_repo")

import ml_dtypes
import numpy as np
import concourse.bass as bass  # noqa: F401
import concourse.tile as tile
from concourse import bacc, mybir
from concourse.bass_utils import run_bass_kernel_spmd

B, CH, CM, CO = 2, 128, 64, 128
H = W = 80
N = H * W                # 6400 key/value pixels per batch element
NCORES = 8
NI = (B * N) // NCORES   # 1600 query pixels per core
JT = N // 128            # 50 key tiles
F32 = mybir.dt.float32
F32R = mybir.dt.float32r
BF16 = mybir.dt.bfloat16

IBS = 400
I_BLOCKS = [(k * IBS, IBS) for k in range(NI // IBS)]
# jt groups of 2: one exp instruction per group
GROUPS = [tuple(range(g, min(g + 2, JT))) for g in range(0, JT, 2)]
NCL = (len(GROUPS) + 3) // 4  # d-sum clusters of 4 groups (8 j-tiles)


def _build(repeat=1):
    """repeat>1 wraps the whole per-core compute in an on-device For_i loop;
    used only by the perf harness to measure HW time via wall-clock slope."""
    nc = bacc.Bacc(None, target_bir_lowering=False)
    zq = nc.declare_dram_parameter("zq", [CH, NI], F32R, isOutput=False)
    zm = nc.declare_dram_parameter("zm", [128, N], F32R, isOutput=False)
    zmb = nc.declare_dram_parameter("zmb", [128, N], BF16, isOutput=False)
    wqk = nc.declare_dram_parameter("wqk", [CH, 128], F32R, isOutput=False)
    bkq = nc.declare_dram_parameter("bkq", [128, 1], F32, isOutput=False)
    wvb = nc.declare_dram_parameter("wvb", [128, CO], BF16, isOutput=False)
    gbv = nc.declare_dram_parameter("gbv", [CO, 1], F32, isOutput=False)
    gsc = nc.declare_dram_parameter("gsc", [1, 1], F32, isOutput=False)
    onesb = nc.declare_dram_parameter("onesb", [128, 1], BF16, isOutput=False)
    out = nc.declare_dram_parameter("out", [CO, NI], F32, isOutput=True)

    with tile.TileContext(nc) as tc:
        with (
            tc.tile_pool(name="big", bufs=1) as big,
            tc.tile_pool(name="expp", bufs=6) as expp,
            tc.tile_pool(name="work", bufs=2) as work,
            tc.tile_pool(name="pse", bufs=2, space="PSUM") as pse,
            tc.tile_pool(name="pspv", bufs=2, space="PSUM") as pspv,
        ):
            zm_sb = big.tile([128, N], F32R)
            nc.sync.dma_start(zm_sb[:], zm[:])
            zmb_sb = big.tile([128, N], BF16)
            nc.sync.dma_start(zmb_sb[:], zmb[:])
            zq_sb = big.tile([CH, NI], F32R)
            nc.sync.dma_start(zq_sb[:], zq[:])
            wqk_sb = big.tile([CH, 128], F32R)
            nc.sync.dma_start(wqk_sb[:], wqk[:])
            bkq_sb = big.tile([128, 1], F32)
            nc.sync.dma_start(bkq_sb[:], bkq[:])
            wvb_sb = big.tile([128, CO], BF16)
            nc.sync.dma_start(wvb_sb[:], wvb[:])
            gbv_sb = big.tile([CO, 1], F32)
            nc.sync.dma_start(gbv_sb[:], gbv[:])
            gsc_sb = big.tile([1, 1], F32)
            nc.sync.dma_start(gsc_sb[:], gsc[:])
            onesb_sb = big.tile([128, 1], BF16)
            nc.sync.dma_start(onesb_sb[:], onesb[:])

            from contextlib import nullcontext
            rep_ctx = tc.For_i(0, repeat, 1) if repeat > 1 else nullcontext()
            with rep_ctx:
                _emit_body(nc, tc, big, expp, work, pse, pspv,
                           zm_sb, zmb_sb, zq_sb, wqk_sb, bkq_sb, wvb_sb,
                           gbv_sb, gsc_sb, onesb_sb, out)

    nc.finalize()
    return nc


def _emit_body(nc, tc, big, expp, work, pse, pspv,
               zm_sb, zmb_sb, zq_sb, wqk_sb, bkq_sb, wvb_sb,
               gbv_sb, gsc_sb, onesb_sb, out):
    # residual (+ folded gamma*bv), exact fp32 bits of z_hsi
    zqp = big.tile([CH, NI], F32)
    nc.vector.tensor_scalar_add(zqp[:], zq_sb[:].bitcast(F32), gbv_sb[:])

    # QK[c, i] = (Wk^T Wq zq + Wk^T bq)[c, i]   -> E = zm^T QK
    QK_sb = big.tile([128, NI], F32R)
    for c0 in range(0, NI, 400):
        pq = pse.tile([128, 1024], F32, tag="e")
        nc.tensor.matmul(pq[:, :400], wqk_sb[:], zq_sb[:, c0:c0 + 400],
                         start=True, stop=True)
        nc.vector.tensor_scalar_add(QK_sb[:, c0:c0 + 400], pq[:, :400],
                                    bkq_sb[:])

    # VT tiles: VT[j, o] = sum_c zm[c, j] Wv[o, c] in bf16 -- computed
    # interleaved into block 0's group loop (quad q lands at group q, always
    # ahead of the lag-3 PV consumer of tiles 2g..2g+1), so ACT's exp chain
    # starts immediately instead of idling behind a PE-only prefix.
    VT_sb = big.tile([128, JT * CO], BF16)

    # main attention loop; PV matmuls trail three exp-groups behind
    for i0, ibs in I_BLOCKS:
        pv = pspv.tile([128, 512], F32, tag="pv")
        dsum = pspv.tile([128, 512], F32, tag="d")
        w_tiles = {}

        def emit_dpv(p3_prev, gi):
            ci = gi // 4
            if gi % 4 == 3 or gi == len(GROUPS) - 1:
                wt = w_tiles.pop(ci)
                nc.tensor.matmul(
                    dsum[:1, :ibs], onesb_sb[:], wt[:, :ibs],
                    start=(ci == 0), stop=(ci == NCL - 1),
                    skip_group_check=True)
            for t, jt in enumerate(GROUPS[gi]):
                nc.tensor.matmul(
                    pv[:, :ibs],
                    VT_sb[:, jt * 128:(jt + 1) * 128],
                    p3_prev[:, t * 512:t * 512 + ibs],
                    start=(jt == 0), stop=(jt == JT - 1),
                    skip_group_check=True)

        from collections import deque
        pending = deque()
        s_hold = u_hold = s_hold2 = None
        for gi, grp in enumerate(GROUPS):
            m = len(grp)
            e3 = pse.tile([128, 1024], F32, tag="e")
            for t, jt in enumerate(grp):
                nc.tensor.matmul(
                    e3[:, t * 512:t * 512 + ibs],
                    zm_sb[:, jt * 128:(jt + 1) * 128],
                    QK_sb[:, i0:i0 + ibs],
                    start=True, stop=True)
            p3 = expp.tile([128, 1024], BF16, tag="p")
            e3v = e3[:].rearrange("p (t x) -> p t x", x=512)[:, :m, :ibs]
            p3v = p3[:].rearrange("p (t x) -> p t x", x=512)[:, :m, :ibs]
            nc.scalar.activation(p3v, e3v, mybir.ActivationFunctionType.Exp)
            s3 = expp.tile([128, 512], BF16, tag="s")
            nc.vector.tensor_add(s3[:, :ibs], p3[:, 0:ibs],
                                 p3[:, 512:512 + ibs])
            # d reduction tree: 8 exp tiles -> one bf16 partial per cluster
            li = gi % 4
            if li == 0:
                s_hold = s3
                if gi == len(GROUPS) - 1:  # ragged last cluster (1 group)
                    w_tiles[gi // 4] = s3
            elif li == 1:
                u_hold = expp.tile([128, 512], BF16, tag="u")
                nc.vector.tensor_add(u_hold[:, :ibs], s_hold[:, :ibs],
                                     s3[:, :ibs])
            elif li == 2:
                s_hold2 = s3
            else:
                u1 = expp.tile([128, 512], BF16, tag="u")
                nc.vector.tensor_add(u1[:, :ibs], s_hold2[:, :ibs],
                                     s3[:, :ibs])
                wt = expp.tile([128, 512], BF16, tag="w")
                nc.vector.tensor_add(wt[:, :ibs], u_hold[:, :ibs],
                                     u1[:, :ibs])
                w_tiles[gi // 4] = wt
            if i0 == 0 and gi * 4 < JT:
                nq = min(4, JT - gi * 4)
                vtq = pspv.tile([128, 512], F32, tag="d")
                for jj in range(nq):
                    j0 = (gi * 4 + jj) * 128
                    nc.tensor.matmul(vtq[:, jj * 128:(jj + 1) * 128],
                                     zmb_sb[:, j0:j0 + 128], wvb_sb[:],
                                     start=True, stop=True)
                nc.vector.tensor_copy(
                    VT_sb[:, gi * 512:gi * 512 + nq * 128],
                    vtq[:, :nq * 128])
            pending.append((p3, gi))
            if len(pending) > 3:
                emit_dpv(*pending.popleft())
        while pending:
            emit_dpv(*pending.popleft())

        # normalize: out = PV * (gamma/d) + zqp
        d_inv = work.tile([1, 512], F32, tag="dinv")
        nc.vector.reciprocal(d_inv[:, :ibs], dsum[:1, :ibs])
        d_g = work.tile([1, 512], F32, tag="dg")
        nc.vector.tensor_scalar_mul(d_g[:, :ibs], d_inv[:, :ibs], gsc_sb[:])
        b_sb = work.tile([128, 512], F32, tag="bsb")
        nc.gpsimd.partition_broadcast(b_sb[:, :ibs], d_g[:1, :ibs])
        t_sb = work.tile([128, 512], F32, tag="tsb")
        nc.vector.tensor_mul(t_sb[:, :ibs], pv[:, :ibs], b_sb[:, :ibs])
        o_sb = work.tile([128, 512], F32, tag="osb")
        nc.vector.tensor_add(o_sb[:, :ibs], t_sb[:, :ibs],
                             zqp[:, i0:i0 + ibs])
        nc.sync.dma_start(out[:, i0:i0 + ibs], o_sb[:, :ibs])


_cached_nc = None


def kernel(z_hsi, z_msi, Wq, bq, Wk, bk, Wv, bv, gamma):
    global _cached_nc
    if _cached_nc is None:
        _cached_nc = _build()
    nc = _cached_nc

    z_hsi = np.asarray(z_hsi, dtype=np.float32).reshape(B, CH, N)
    z_msi = np.ascontiguousarray(np.asarray(z_msi, dtype=np.float32).reshape(B, CM, N))
    Wq64 = np.asarray(Wq, dtype=np.float64)
    Wk64 = np.asarray(Wk, dtype=np.float64)
    bq64 = np.asarray(bq, dtype=np.float64)
    # QK folding: E = zm^T (Wk^T Wq zq + Wk^T bq); bk cancels in softmax.
    # All CM=64 contractions are zero-padded to 128: K=64 matmuls run ~2x
    # slower per column on TRN2 than K=128.
    wqk_h = np.zeros((CH, 128), np.float32)
    wqk_h[:, :CM] = (Wq64.T @ Wk64).astype(np.float32)
    bkq_h = np.zeros((128, 1), np.float32)
    bkq_h[:CM, 0] = (Wk64.T @ bq64).astype(np.float32)
    wvb_h = np.zeros((128, CO), ml_dtypes.bfloat16)
    wvb_h[:CM] = np.asarray(Wv, np.float32).T.astype(ml_dtypes.bfloat16)
    z_msi_pad = np.zeros((B, 128, N), np.float32)
    z_msi_pad[:, :CM] = z_msi
    zmb_h = z_msi_pad.astype(ml_dtypes.bfloat16)
    g = float(np.asarray(gamma, dtype=np.float32).reshape(-1)[0])
    gbv = np.ascontiguousarray((g * np.asarray(bv, np.float32)).reshape(CO, 1))
    gsc = np.full((1, 1), g, dtype=np.float32)
    onesb = np.ones((128, 1), dtype=ml_dtypes.bfloat16)

    shards_per_b = NCORES // B
    in_maps = []
    for c in range(NCORES):
        b, s = c // shards_per_b, (c % shards_per_b) * NI
        in_maps.append({
            "zq": np.ascontiguousarray(z_hsi[b][:, s:s + NI]),
            "zm": z_msi_pad[b],
            "zmb": zmb_h[b],
            "wqk": wqk_h, "bkq": bkq_h, "wvb": wvb_h,
            "gbv": gbv, "gsc": gsc, "onesb": onesb,
        })

    res = run_bass_kernel_spmd(nc, in_maps, core_ids=list(range(NCORES)))

    out = np.empty((B, CH, N), dtype=np.float32)
    for c in range(NCORES):
        b, s = c // shards_per_b, (c % shards_per_b) * NI
        out[b][:, s:s + NI] = res.results[c]["out"]
    return out.reshape(B, CH, H, W)
